# revision 16
# baseline (speedup 1.0000x reference)
"""Trainium2 Bass kernel: MultiHeadSelfAttention (B=1, S=4096, D=512, H=8, DK=DV=64)
with fc_out applied twice.

Sharding: sequence-sharded across 8 cores (512 queries per core). Every core
receives the FULL keys/values (pre-transposed, bf16) and redundantly computes
the full K^T / V projections on-device (cheaper than an AllGather, whose entry
barrier + transfer measured ~100us); attention + the two output projections run
on the core's own 512-query chunk. Host concatenates the 8 output chunks.

Layout notes:
  - scores^T tiles [seq_k(128) x seq_q(512)] come out of PE via lhsT=K^T block,
    rhs=q^T. Both are zero-padded from d=64 to K=128 partitions: K=64 matmuls
    never trip the PE HAM activity monitor, pinning the clock to 1.2 GHz
    (measured); K=128 with zero rows sustains 2.4 GHz.
  - softmax denominator via a ones-column appended to each head's V (stride
    65): attn@V gives [65, 512] per head = output^T rows + exp-sum row.
"""
import sys, functools
sys.path.insert(0, "/opt/trn_rl_repo")
if "/root/.axon_site" not in sys.path:
    sys.path.insert(0, "/root/.axon_site")
import numpy as np
import ml_dtypes

import concourse.bass as bass
import concourse.tile as tile
from concourse import bacc, mybir, masks
from concourse.bass_utils import run_bass_kernel_spmd

NCORES = 8
S, D, H, DK = 4096, 512, 8, 64
CH = S // NCORES            # 512 sequence rows per core
VW = H * (DK + 1)           # 520: v row width incl. ones columns
JT = S // 128               # 32 seq_k tiles
CHUNK = 3                   # j-tiles per exp batch ([128,1536] psum, 3 banks x 2)

F32 = mybir.dt.float32
BF16 = mybir.dt.bfloat16
EXP = mybir.ActivationFunctionType.Exp


def _build_program():
    nc = bacc.Bacc("TRN2", target_bir_lowering=False, debug=False,
                   num_devices=NCORES)

    xqT = nc.dram_tensor("xqT", [D, CH], BF16, kind="ExternalInput")
    keysT = nc.dram_tensor("keysT", [D, S], BF16, kind="ExternalInput")
    valsT = nc.dram_tensor("valsT", [D, S], BF16, kind="ExternalInput")
    Wq = nc.dram_tensor("Wq", [D, D], BF16, kind="ExternalInput")
    Wk = nc.dram_tensor("Wk", [D, D], BF16, kind="ExternalInput")
    Wv = nc.dram_tensor("Wv", [D, D], BF16, kind="ExternalInput")
    Wo = nc.dram_tensor("Wo", [D, D], BF16, kind="ExternalInput")
    bo = nc.dram_tensor("bo", [D], F32, kind="ExternalInput")
    y = nc.dram_tensor("y", [CH, D], F32, kind="ExternalOutput")

    with tile.TileContext(nc) as tc:
        with tc.tile_pool(name="persist", bufs=1) as pp, \
             tc.tile_pool(name="kv", bufs=1) as kvp:

            Wo_sb = pp.tile([128, 2048], BF16, tag="wo")
            Wk_sb = pp.tile([128, 2048], BF16, tag="wk")
            Wv_sb = pp.tile([128, 2048], BF16, tag="wv")
            bo_sb = pp.tile([128, 4], F32, tag="bo")
            ident = pp.tile([128, 128], BF16, tag="id")
            # q^T per head: even heads rows 0-63 (zeros below), odd heads rows
            # 64-127 (zeros above) - matches the packed K^T pair layout
            qTz_sb = pp.tile([128, H * 512], BF16, tag="qt")
            attT_sb = pp.tile([128, 2048], BF16, tag="att")
            o1acc = pp.tile([128, 2048], F32, tag="o1a")
            # K^T packed head pairs: head 2p on rows 0-63, head 2p+1 on 64-127;
            # the zero padding that keeps scores at K=128 lives in qTz instead
            KTp = [kvp.tile([128, S], BF16, tag=f"kt{p}", name=f"KT{p}")
                   for p in range(H // 2)]
            # V natural [seq, head-stripes of 65 (64 + ones col)]
            V_sb = kvp.tile([128, JT * VW], BF16, tag="v")

            masks.make_identity(nc, ident[:])
            # zero pads + ones columns on gpsimd (keeps DVE free)
            nc.vector.memset(qTz_sb[:], 0.0)
            nc.gpsimd.memset(
                V_sb[:].rearrange("p (j h x) -> p j h x", j=JT, h=H, x=DK + 1)
                [:, :, :, DK:DK + 1], 1.0)

            with tc.tile_pool(name="kstage", bufs=1) as ksp, \
                 tc.tile_pool(name="pt", bufs=6) as ptp, \
                 tc.tile_pool(name="rc", bufs=2) as rcp, \
                 tc.tile_pool(name="ps_av", bufs=1, space="PSUM") as psav:

                def staged_load(dst_sb, src_dram, nchunks=8):
                    w = S // nchunks
                    dst = dst_sb[:].rearrange("p (k s) -> p k s", k=4)
                    srcv = src_dram.ap().rearrange("(k p) s -> p k s", p=128)
                    for ci in range(nchunks):
                        nc.sync.dma_start(dst[:, :, w * ci:w * ci + w],
                                          srcv[:, :, w * ci:w * ci + w])

                def q_proj(pool):
                    for m in range(4):
                        ps = pool.tile([128, 512], F32, tag="bg", name=f"qp{m}")
                        for k in range(4):
                            nc.tensor.matmul(
                                ps[:], lhsT=Wq_sb[:, 512 * k + 128 * m:512 * k + 128 * m + 128],
                                rhs=xqT_sb[:, 512 * k:512 * k + 512],
                                start=(k == 0), stop=(k == 3))
                        nc.vector.tensor_copy(
                            qTz_sb[0:64, 512 * (2 * m):512 * (2 * m) + 512], ps[0:64, :])
                        nc.vector.tensor_copy(
                            qTz_sb[64:128, 512 * (2 * m + 1):512 * (2 * m + 1) + 512],
                            ps[64:128, :])

                def v_proj_group(j, pool):
                    ps = pool.tile([128, 512], F32, tag="bg", name=f"vp{j}")
                    for k in range(4):
                        nc.tensor.matmul(
                            ps[:], lhsT=vst[:, S * k + 128 * j:S * k + 128 * j + 128],
                            rhs=Wv_sb[:, 512 * k:512 * k + 512],
                            start=(k == 0), stop=(k == 3))
                    dst = V_sb[:, VW * j:VW * j + VW].rearrange(
                        "p (h x) -> p h x", h=H, x=DK + 1)[:, :, 0:DK]
                    nc.vector.tensor_copy(
                        dst, ps[:].rearrange("p (h x) -> p h x", h=H, x=DK))

                def k_proj_group(m, sc, kst, pool, tag):
                    ps = pool.tile([128, 512], F32, tag=tag, name=f"kp{m}_{sc}")
                    for k in range(4):
                        nc.tensor.matmul(
                            ps[:], lhsT=Wk_sb[:, 512 * k + 128 * m:512 * k + 128 * m + 128],
                            rhs=kst[:, S * k + 512 * sc:S * k + 512 * sc + 512],
                            start=(k == 0), stop=(k == 3))
                    nc.vector.tensor_copy(KTp[m][:, 512 * sc:512 * sc + 512], ps[:])

                def fc1_partial_m(hp, m, pool, tag):
                    # out1 partial: pair hp's feature block through Wo (one m-tile)
                    ps = pool.tile([128, 512], F32, tag=tag, name=f"f1{hp}_{m}")
                    nc.tensor.matmul(
                        ps[:], lhsT=Wo_sb[:, 512 * hp + 128 * m:512 * hp + 128 * m + 128],
                        rhs=attT_sb[:, 512 * hp:512 * hp + 512],
                        start=True, stop=True)
                    if hp == 0:
                        nc.vector.tensor_scalar_add(
                            o1acc[:, 512 * m:512 * m + 512], ps[:], bo_sb[:, m:m + 1])
                    else:
                        nc.vector.tensor_add(
                            o1acc[:, 512 * m:512 * m + 512],
                            o1acc[:, 512 * m:512 * m + 512], ps[:])

                def make_drip_fc(hp):
                    def drip(c):
                        if c < 4:
                            fc1_partial_m(hp, c, pskp, "kp")
                    return drip

                def attention_head(h, pool_sc, chunk, drip=None):
                    q_ap = qTz_sb[:, 512 * h:512 * h + 512]
                    av = psav.tile([65, 512], F32, tag="av", name=f"av{h}")

                    def attn_v(js, pt):
                        for i, j in enumerate(js):
                            nc.tensor.matmul(
                                av[:],
                                lhsT=V_sb[:, VW * j + 65 * h:VW * j + 65 * h + 65],
                                rhs=pt[:, 512 * i:512 * i + 512],
                                start=(j == 0), stop=(j == JT - 1))

                    pend = None  # attn@V lags one chunk so scores stay ahead of ACT
                    for c in range((JT + chunk - 1) // chunk):
                        js = list(range(chunk * c, min(chunk * c + chunk, JT)))
                        ps = pool_sc.tile([128, 512 * chunk], F32, tag="sc",
                                          name=f"sc{h}_{c}")
                        pt = ptp.tile([128, 512 * chunk], BF16, tag="pt",
                                      name=f"pt{h}_{c}")
                        for i, j in enumerate(js):
                            nc.tensor.matmul(
                                ps[:, 512 * i:512 * i + 512],
                                lhsT=KTp[h // 2][:, 128 * j:128 * j + 128],
                                rhs=q_ap, start=True, stop=True)
                        w = 512 * len(js)
                        nc.scalar.activation(pt[:, 0:w], ps[:, 0:w], EXP, scale=0.125)
                        if drip is not None:
                            drip(c)
                        if pend is not None:
                            attn_v(*pend)
                        pend = (js, pt)
                    attn_v(*pend)
                    hp, hl = h // 2, h % 2
                    # copy psum accumulator out immediately so the bank frees
                    avc = rcp.tile([65, 512], F32, tag="avc", name=f"avc{h}")
                    rbc = rcp.tile([64, 512], F32, tag="rb", name=f"rb{h}")
                    rtmp = rcp.tile([1, 512], F32, tag="rt", name=f"rt{h}")
                    nc.vector.tensor_copy(avc[:], av[:])
                    nc.vector.tensor_copy(rtmp[:], av[64:65, :])
                    rb2 = rcp.tile([64, 512], F32, tag="rb2", name=f"rb2{h}")
                    nc.gpsimd.partition_broadcast(rbc[:], rtmp[:])
                    nc.vector.reciprocal_approx_fast(out=rb2[:], in_=rbc[:])
                    nc.vector.tensor_mul(
                        attT_sb[64 * hl:64 * hl + 64, 512 * hp:512 * hp + 512],
                        avc[0:64, :], rb2[:])

                # ---- scope A: q proj, K0, head 0 with V-proj dripped in ----
                pfx = tc.tile_pool(name="xin", bufs=1)
                xp = pfx.__enter__()
                scA = tc.tile_pool(name="ps_scA", bufs=2, space="PSUM")
                pssc2 = scA.__enter__()
                bgp_cm = tc.tile_pool(name="ps_bg", bufs=2, space="PSUM")
                bgp = bgp_cm.__enter__()

                Wq_sb = xp.tile([128, 2048], BF16, tag="wq")
                xqT_sb = xp.tile([128, 2048], BF16, tag="xq")
                vst = xp.tile([128, 4 * S], BF16, tag="vs", name="vstage")
                kst0 = ksp.tile([128, 4 * S], BF16, tag="ks", name="kstage0")
                nc.sync.dma_start(
                    xqT_sb[:].rearrange("p (k s) -> p k s", k=4),
                    xqT.ap().rearrange("(k p) s -> p k s", p=128))
                nc.sync.dma_start(
                    Wq_sb[:].rearrange("p (k n) -> p k n", k=4),
                    Wq.ap().rearrange("(k p) n -> p k n", p=128))
                nc.sync.dma_start(
                    Wk_sb[:].rearrange("p (k n) -> p k n", k=4),
                    Wk.ap().rearrange("(k p) n -> p k n", p=128))
                staged_load(kst0, keysT)
                nc.sync.dma_start(
                    Wv_sb[:].rearrange("p (k n) -> p k n", k=4),
                    Wv.ap().rearrange("(k p) n -> p k n", p=128))
                staged_load(vst, valsT)
                nc.sync.dma_start(
                    Wo_sb[:].rearrange("p (k n) -> p k n", k=4),
                    Wo.ap().rearrange("(k p) n -> p k n", p=128))
                nc.sync.dma_start(bo_sb[:], bo.ap().rearrange("(m p) -> p m", p=128))

                q_proj(bgp)
                for sc in range(8):
                    k_proj_group(0, sc, kst0, bgp, "bg")
                kst1 = ksp.tile([128, 4 * S], BF16, tag="ks", name="kstage1")
                staged_load(kst1, keysT)
                v_proj_group(0, bgp)
                v_proj_group(1, bgp)

                def drip_v(c):
                    for j in (2 * c + 2, 2 * c + 3):
                        if j < JT:
                            v_proj_group(j, bgp)

                attention_head(0, pssc2, 2, drip_v)

                bgp_cm.__exit__(None, None, None)
                scA.__exit__(None, None, None)
                pfx.__exit__(None, None, None)

                # ---- scope B: heads 1-7, K pairs 1-3 dripped in ----
                with tc.tile_pool(name="ps_sc", bufs=2, space="PSUM") as pssc, \
                     tc.tile_pool(name="ps_kp", bufs=1, space="PSUM") as pskp:
                    ksts = {1: kst1}

                    def make_drip_k(m):
                        def drip(c):
                            if c < 8:
                                k_proj_group(m, c, ksts[m], pskp, "kp")
                        return drip

                    attention_head(1, pssc, CHUNK, make_drip_k(1))
                    ksts[2] = ksp.tile([128, 4 * S], BF16, tag="ks", name="kstage2")
                    staged_load(ksts[2], keysT)
                    attention_head(2, pssc, CHUNK, make_drip_fc(0))
                    attention_head(3, pssc, CHUNK, make_drip_k(2))
                    ksts[3] = ksp.tile([128, 4 * S], BF16, tag="ks", name="kstage3")
                    staged_load(ksts[3], keysT)
                    attention_head(4, pssc, CHUNK, make_drip_fc(1))
                    attention_head(5, pssc, CHUNK, make_drip_k(3))
                    attention_head(6, pssc, CHUNK, make_drip_fc(2))
                    attention_head(7, pssc, CHUNK)
                    for m in range(4):
                        fc1_partial_m(3, m, pskp, "kp")

            # ---- fc_out twice (fp32), then transpose to natural layout ----
            with tc.tile_pool(name="fc", bufs=3) as fcp, \
                 tc.tile_pool(name="ps_fc", bufs=2, space="PSUM") as psfc, \
                 tc.tile_pool(name="ps_tr", bufs=2, space="PSUM") as pstr:
                o1T = fcp.tile([128, 2048], BF16, tag="fcb", name="o1T")
                o2T = fcp.tile([128, 2048], BF16, tag="fcb", name="o2T")
                nc.vector.tensor_copy(o1T[:], o1acc[:])
                for m in range(4):
                    ps = psfc.tile([128, 512], F32, tag="fc")
                    for k in range(4):
                        nc.tensor.matmul(
                            ps[:], lhsT=Wo_sb[:, 512 * k + 128 * m:512 * k + 128 * m + 128],
                            rhs=o1T[:, 512 * k:512 * k + 512],
                            start=(k == 0), stop=(k == 3))
                    nc.vector.tensor_scalar_add(
                        o2T[:, 512 * m:512 * m + 512], ps[:], bo_sb[:, m:m + 1])
                onat = fcp.tile([128, 2048], F32, tag="fcb", name="onat")
                for kf in range(4):
                    for m in range(4):
                        pst = pstr.tile([128, 128], BF16, tag="tr")
                        nc.tensor.transpose(
                            pst[:], o2T[:, 512 * kf + 128 * m:512 * kf + 128 * m + 128],
                            ident[:])
                        nc.vector.tensor_copy(
                            onat[:, 512 * m + 128 * kf:512 * m + 128 * kf + 128],
                            pst[:])
                nc.sync.dma_start(
                    y.ap().rearrange("(m p) f -> p m f", m=4, p=128),
                    onat[:].rearrange("p (m f) -> p m f", m=4))

    nc.compile()
    return nc


@functools.lru_cache(maxsize=1)
def _get_program():
    return _build_program()


def _make_in_maps(queries, keys, values, Wq, Wk, Wv, Wo, bo):
    q = np.asarray(queries, np.float32).reshape(S, D)
    kT = np.ascontiguousarray(np.asarray(keys, np.float32).reshape(S, D).T
                              ).astype(ml_dtypes.bfloat16)
    vT = np.ascontiguousarray(np.asarray(values, np.float32).reshape(S, D).T
                              ).astype(ml_dtypes.bfloat16)
    Wq = np.ascontiguousarray(np.asarray(Wq, np.float32)).astype(ml_dtypes.bfloat16)
    Wk = np.ascontiguousarray(np.asarray(Wk, np.float32)).astype(ml_dtypes.bfloat16)
    Wv = np.ascontiguousarray(np.asarray(Wv, np.float32)).astype(ml_dtypes.bfloat16)
    Wo = np.ascontiguousarray(np.asarray(Wo, np.float32)).astype(ml_dtypes.bfloat16)
    bo = np.ascontiguousarray(np.asarray(bo, np.float32))
    in_maps = []
    for c in range(NCORES):
        in_maps.append({
            "xqT": np.ascontiguousarray(q[c * CH:(c + 1) * CH].T).astype(ml_dtypes.bfloat16),
            "keysT": kT, "valsT": vT,
            "Wq": Wq, "Wk": Wk, "Wv": Wv, "Wo": Wo, "bo": bo,
        })
    return in_maps


def _run(in_maps, **kw):
    nc = _get_program()
    return run_bass_kernel_spmd(nc, in_maps, core_ids=list(range(NCORES)), **kw)


def kernel(queries, keys, values, Wq, Wk, Wv, Wo, bo):
    res = _run(_make_in_maps(queries, keys, values, Wq, Wk, Wv, Wo, bo))
    out = np.concatenate([res.results[c]["y"] for c in range(NCORES)], axis=0)
    return out.reshape(1, S, D)


def run_traced(queries, keys, values, Wq, Wk, Wv, Wo, bo):
    """Like kernel() but with NTFF profiling; returns (output, BassKernelResults)."""
    import types
    import trn_agent_boot.trn_boot as _tb
    from concourse import bass_utils
    hook = _tb._ntff_profile_via_ctypes("/opt/axon/libaxon_pjrt.so")
    mod = types.ModuleType("antenv.axon_hooks")
    mod.get_axon_ntff_profile_hook = lambda: hook
    sys.modules["antenv.axon_hooks"] = mod
    bass_utils.upload_artifacts = lambda tmpdir: tmpdir
    res = _run(_make_in_maps(queries, keys, values, Wq, Wk, Wv, Wo, bo), trace=True)
    out = np.concatenate([res.results[c]["y"] for c in range(NCORES)], axis=0)
    return out.reshape(1, S, D), res


# revision 17
# speedup vs baseline: 1.0562x; 1.0562x over previous
"""Trainium2 Bass kernel: MultiHeadSelfAttention (B=1, S=4096, D=512, H=8, DK=DV=64)
with fc_out applied twice.

Sharding: sequence-sharded across 8 cores (512 queries per core). Every core
receives the FULL keys/values (pre-transposed, bf16) and redundantly computes
the full K^T / V projections on-device (cheaper than an AllGather, whose entry
barrier + transfer measured ~100us); attention + the two output projections run
on the core's own 512-query chunk. Host concatenates the 8 output chunks.

Layout notes:
  - scores^T tiles [seq_k(128) x seq_q(512)] come out of PE via lhsT=K^T block,
    rhs=q^T. Both are zero-padded from d=64 to K=128 partitions: K=64 matmuls
    never trip the PE HAM activity monitor, pinning the clock to 1.2 GHz
    (measured); K=128 with zero rows sustains 2.4 GHz.
  - softmax denominator via a ones-column appended to each head's V (stride
    65): attn@V gives [65, 512] per head = output^T rows + exp-sum row.
"""
import sys, functools
sys.path.insert(0, "/opt/trn_rl_repo")
if "/root/.axon_site" not in sys.path:
    sys.path.insert(0, "/root/.axon_site")
import numpy as np
import ml_dtypes

import concourse.bass as bass
import concourse.tile as tile
from concourse import bacc, mybir, masks
from concourse.bass_utils import run_bass_kernel_spmd

NCORES = 8
S, D, H, DK = 4096, 512, 8, 64
CH = S // NCORES            # 512 sequence rows per core
VW = H * (DK + 1)           # 520: v row width incl. ones columns
JT = S // 128               # 32 seq_k tiles
CHUNK = 3                   # j-tiles per exp batch ([128,1536] psum, 3 banks x 2)

F32 = mybir.dt.float32
BF16 = mybir.dt.bfloat16
EXP = mybir.ActivationFunctionType.Exp


def _build_program():
    nc = bacc.Bacc("TRN2", target_bir_lowering=False, debug=False,
                   num_devices=NCORES)

    xqT = nc.dram_tensor("xqT", [D, CH], BF16, kind="ExternalInput")
    keysT = nc.dram_tensor("keysT", [D, S], BF16, kind="ExternalInput")
    valsT = nc.dram_tensor("valsT", [D, S], BF16, kind="ExternalInput")
    Wq = nc.dram_tensor("Wq", [D, D], BF16, kind="ExternalInput")
    Wk = nc.dram_tensor("Wk", [D, D], BF16, kind="ExternalInput")
    Wv = nc.dram_tensor("Wv", [D, D], BF16, kind="ExternalInput")
    Wo = nc.dram_tensor("Wo", [D, D], BF16, kind="ExternalInput")
    bo = nc.dram_tensor("bo", [D], F32, kind="ExternalInput")
    y = nc.dram_tensor("y", [CH, D], F32, kind="ExternalOutput")

    with tile.TileContext(nc) as tc:
        with tc.tile_pool(name="persist", bufs=1) as pp, \
             tc.tile_pool(name="kv", bufs=1) as kvp:

            Wo_sb = pp.tile([128, 2048], BF16, tag="wo")
            Wk_sb = pp.tile([128, 2048], BF16, tag="wk")
            Wv_sb = pp.tile([128, 2048], BF16, tag="wv")
            bo_sb = pp.tile([128, 4], F32, tag="bo")
            ident = pp.tile([128, 128], BF16, tag="id")
            # q^T per head: even heads rows 0-63 (zeros below), odd heads rows
            # 64-127 (zeros above) - matches the packed K^T pair layout
            qTz_sb = pp.tile([128, H * 512], BF16, tag="qt")
            attT_sb = pp.tile([128, 2048], BF16, tag="att")
            o1acc = pp.tile([128, 2048], F32, tag="o1a")
            # K^T packed head pairs: head 2p on rows 0-63, head 2p+1 on 64-127;
            # the zero padding that keeps scores at K=128 lives in qTz instead
            KTp = [kvp.tile([128, S], BF16, tag=f"kt{p}", name=f"KT{p}")
                   for p in range(H // 2)]
            # V natural [seq, head-stripes of 65 (64 + ones col)]
            V_sb = kvp.tile([128, JT * VW], BF16, tag="v")

            masks.make_identity(nc, ident[:])
            # zero pads + ones columns on gpsimd (keeps DVE free)
            nc.vector.memset(qTz_sb[:], 0.0)
            nc.gpsimd.memset(
                V_sb[:].rearrange("p (j h x) -> p j h x", j=JT, h=H, x=DK + 1)
                [:, :, :, DK:DK + 1], 1.0)

            with tc.tile_pool(name="kstage", bufs=1) as ksp, \
                 tc.tile_pool(name="pt", bufs=6) as ptp, \
                 tc.tile_pool(name="rc", bufs=2) as rcp, \
                 tc.tile_pool(name="ps_av", bufs=1, space="PSUM") as psav:

                def staged_load(dst_sb, src_dram, nchunks=8):
                    w = S // nchunks
                    dst = dst_sb[:].rearrange("p (k s) -> p k s", k=4)
                    srcv = src_dram.ap().rearrange("(k p) s -> p k s", p=128)
                    for ci in range(nchunks):
                        nc.sync.dma_start(dst[:, :, w * ci:w * ci + w],
                                          srcv[:, :, w * ci:w * ci + w])

                def q_proj(pool):
                    for m in range(4):
                        ps = pool.tile([128, 512], F32, tag="bg", name=f"qp{m}")
                        for k in range(4):
                            nc.tensor.matmul(
                                ps[:], lhsT=Wq_sb[:, 512 * k + 128 * m:512 * k + 128 * m + 128],
                                rhs=xqT_sb[:, 512 * k:512 * k + 512],
                                start=(k == 0), stop=(k == 3))
                        nc.vector.tensor_copy(
                            qTz_sb[0:64, 512 * (2 * m):512 * (2 * m) + 512], ps[0:64, :])
                        nc.vector.tensor_copy(
                            qTz_sb[64:128, 512 * (2 * m + 1):512 * (2 * m + 1) + 512],
                            ps[64:128, :])

                def v_proj_group(j, pool):
                    ps = pool.tile([128, 512], F32, tag="bg", name=f"vp{j}")
                    for k in range(4):
                        nc.tensor.matmul(
                            ps[:], lhsT=vst[:, S * k + 128 * j:S * k + 128 * j + 128],
                            rhs=Wv_sb[:, 512 * k:512 * k + 512],
                            start=(k == 0), stop=(k == 3))
                    dst = V_sb[:, VW * j:VW * j + VW].rearrange(
                        "p (h x) -> p h x", h=H, x=DK + 1)[:, :, 0:DK]
                    nc.vector.tensor_copy(
                        dst, ps[:].rearrange("p (h x) -> p h x", h=H, x=DK))

                def k_proj_group(m, sc, kst, pool, tag):
                    ps = pool.tile([128, 512], F32, tag=tag, name=f"kp{m}_{sc}")
                    for k in range(4):
                        nc.tensor.matmul(
                            ps[:], lhsT=Wk_sb[:, 512 * k + 128 * m:512 * k + 128 * m + 128],
                            rhs=kst[:, S * k + 512 * sc:S * k + 512 * sc + 512],
                            start=(k == 0), stop=(k == 3))
                    nc.vector.tensor_copy(KTp[m][:, 512 * sc:512 * sc + 512], ps[:])

                def fc1_partial_m(hp, m, pool, tag):
                    # out1 partial: pair hp's feature block through Wo (one m-tile)
                    ps = pool.tile([128, 512], F32, tag=tag, name=f"f1{hp}_{m}")
                    nc.tensor.matmul(
                        ps[:], lhsT=Wo_sb[:, 512 * hp + 128 * m:512 * hp + 128 * m + 128],
                        rhs=attT_sb[:, 512 * hp:512 * hp + 512],
                        start=True, stop=True)
                    if hp == 0:
                        nc.vector.tensor_scalar_add(
                            o1acc[:, 512 * m:512 * m + 512], ps[:], bo_sb[:, m:m + 1])
                    else:
                        nc.vector.tensor_add(
                            o1acc[:, 512 * m:512 * m + 512],
                            o1acc[:, 512 * m:512 * m + 512], ps[:])

                def make_drip_fc(hp):
                    def drip(c):
                        if c < 4:
                            fc1_partial_m(hp, c, pskp, "kp")
                    return drip

                def attention_head(h, pool_sc, chunk, drip=None):
                    q_ap = qTz_sb[:, 512 * h:512 * h + 512]
                    av = psav.tile([65, 512], F32, tag="av", name=f"av{h}")

                    def attn_v(js, pt):
                        for i, j in enumerate(js):
                            nc.tensor.matmul(
                                av[:],
                                lhsT=V_sb[:, VW * j + 65 * h:VW * j + 65 * h + 65],
                                rhs=pt[:, 512 * i:512 * i + 512],
                                start=(j == 0), stop=(j == JT - 1))

                    pend = None  # attn@V lags one chunk so scores stay ahead of ACT
                    for c in range((JT + chunk - 1) // chunk):
                        js = list(range(chunk * c, min(chunk * c + chunk, JT)))
                        ps = pool_sc.tile([128, 512 * chunk], F32, tag="sc",
                                          name=f"sc{h}_{c}")
                        pt = ptp.tile([128, 512 * chunk], BF16, tag="pt",
                                      name=f"pt{h}_{c}")
                        for i, j in enumerate(js):
                            nc.tensor.matmul(
                                ps[:, 512 * i:512 * i + 512],
                                lhsT=KTp[h // 2][:, 128 * j:128 * j + 128],
                                rhs=q_ap, start=True, stop=True)
                        w = 512 * len(js)
                        nc.scalar.activation(pt[:, 0:w], ps[:, 0:w], EXP, scale=0.125)
                        if drip is not None:
                            drip(c)
                        if pend is not None:
                            attn_v(*pend)
                        pend = (js, pt)
                    attn_v(*pend)
                    hp, hl = h // 2, h % 2
                    # copy psum accumulator out immediately so the bank frees
                    avc = rcp.tile([65, 512], F32, tag="avc", name=f"avc{h}")
                    rbc = rcp.tile([64, 512], F32, tag="rb", name=f"rb{h}")
                    rtmp = rcp.tile([1, 512], F32, tag="rt", name=f"rt{h}")
                    nc.vector.tensor_copy(avc[:], av[:])
                    nc.vector.tensor_copy(rtmp[:], av[64:65, :])
                    rb2 = rcp.tile([64, 512], F32, tag="rb2", name=f"rb2{h}")
                    nc.gpsimd.partition_broadcast(rbc[:], rtmp[:])
                    nc.vector.reciprocal_approx_fast(out=rb2[:], in_=rbc[:])
                    nc.vector.tensor_mul(
                        attT_sb[64 * hl:64 * hl + 64, 512 * hp:512 * hp + 512],
                        avc[0:64, :], rb2[:])

                # ---- scope A: q proj, K0, head 0 with V-proj dripped in ----
                pfx = tc.tile_pool(name="xin", bufs=1)
                xp = pfx.__enter__()
                scA = tc.tile_pool(name="ps_scA", bufs=2, space="PSUM")
                pssc2 = scA.__enter__()
                bgp_cm = tc.tile_pool(name="ps_bg", bufs=2, space="PSUM")
                bgp = bgp_cm.__enter__()

                Wq_sb = xp.tile([128, 2048], BF16, tag="wq")
                xqT_sb = xp.tile([128, 2048], BF16, tag="xq")
                vst = xp.tile([128, 4 * S], BF16, tag="vs", name="vstage")
                kst0 = ksp.tile([128, 4 * S], BF16, tag="ks", name="kstage0")
                nc.sync.dma_start(
                    xqT_sb[:].rearrange("p (k s) -> p k s", k=4),
                    xqT.ap().rearrange("(k p) s -> p k s", p=128))
                nc.sync.dma_start(
                    Wq_sb[:].rearrange("p (k n) -> p k n", k=4),
                    Wq.ap().rearrange("(k p) n -> p k n", p=128))
                nc.sync.dma_start(
                    Wk_sb[:].rearrange("p (k n) -> p k n", k=4),
                    Wk.ap().rearrange("(k p) n -> p k n", p=128))
                staged_load(kst0, keysT)
                nc.sync.dma_start(
                    Wv_sb[:].rearrange("p (k n) -> p k n", k=4),
                    Wv.ap().rearrange("(k p) n -> p k n", p=128))
                staged_load(vst, valsT)
                nc.sync.dma_start(
                    Wo_sb[:].rearrange("p (k n) -> p k n", k=4),
                    Wo.ap().rearrange("(k p) n -> p k n", p=128))
                nc.sync.dma_start(bo_sb[:], bo.ap().rearrange("(m p) -> p m", p=128))

                q_proj(bgp)
                for sc in range(8):
                    k_proj_group(0, sc, kst0, bgp, "bg")
                kst1 = ksp.tile([128, 4 * S], BF16, tag="ks", name="kstage1")
                staged_load(kst1, keysT)
                v_proj_group(0, bgp)
                v_proj_group(1, bgp)

                def drip_v(c):
                    for j in (2 * c + 2, 2 * c + 3):
                        if j < JT:
                            v_proj_group(j, bgp)

                attention_head(0, pssc2, 2, drip_v)

                bgp_cm.__exit__(None, None, None)
                scA.__exit__(None, None, None)
                pfx.__exit__(None, None, None)

                # ---- scope B: heads 1-7, K pairs 1-3 dripped in ----
                with tc.tile_pool(name="ps_sc", bufs=2, space="PSUM") as pssc, \
                     tc.tile_pool(name="ps_kp", bufs=1, space="PSUM") as pskp:
                    ksts = {1: kst1}

                    def make_drip_k(m):
                        def drip(c):
                            if c < 8:
                                k_proj_group(m, c, ksts[m], pskp, "kp")
                        return drip

                    attention_head(1, pssc, CHUNK, make_drip_k(1))
                    ksts[2] = ksp.tile([128, 4 * S], BF16, tag="ks", name="kstage2")
                    staged_load(ksts[2], keysT)
                    attention_head(2, pssc, CHUNK)
                    attention_head(3, pssc, CHUNK, make_drip_k(2))
                    ksts[3] = ksp.tile([128, 4 * S], BF16, tag="ks", name="kstage3")
                    staged_load(ksts[3], keysT)
                    attention_head(4, pssc, CHUNK)
                    attention_head(5, pssc, CHUNK, make_drip_k(3))
                    attention_head(6, pssc, CHUNK)
                    attention_head(7, pssc, CHUNK)

            # ---- fc_out twice (fp32), then transpose to natural layout ----
            with tc.tile_pool(name="fc", bufs=3) as fcp, \
                 tc.tile_pool(name="ps_fc", bufs=2, space="PSUM") as psfc, \
                 tc.tile_pool(name="ps_tr", bufs=2, space="PSUM") as pstr:
                o1T = fcp.tile([128, 2048], BF16, tag="fcb", name="o1T")
                o2T = fcp.tile([128, 2048], BF16, tag="fcb", name="o2T")
                for fsrc, fdst in ((attT_sb, o1T), (o1T, o2T)):
                    for m in range(4):
                        ps = psfc.tile([128, 512], F32, tag="fc")
                        for k in range(4):
                            nc.tensor.matmul(
                                ps[:], lhsT=Wo_sb[:, 512 * k + 128 * m:512 * k + 128 * m + 128],
                                rhs=fsrc[:, 512 * k:512 * k + 512],
                                start=(k == 0), stop=(k == 3))
                        nc.vector.tensor_scalar_add(
                            fdst[:, 512 * m:512 * m + 512], ps[:], bo_sb[:, m:m + 1])
                onat = fcp.tile([128, 2048], F32, tag="fcb", name="onat")
                for kf in range(4):
                    for m in range(4):
                        pst = pstr.tile([128, 128], BF16, tag="tr")
                        nc.tensor.transpose(
                            pst[:], o2T[:, 512 * kf + 128 * m:512 * kf + 128 * m + 128],
                            ident[:])
                        nc.vector.tensor_copy(
                            onat[:, 512 * m + 128 * kf:512 * m + 128 * kf + 128],
                            pst[:])
                nc.sync.dma_start(
                    y.ap().rearrange("(m p) f -> p m f", m=4, p=128),
                    onat[:].rearrange("p (m f) -> p m f", m=4))

    nc.compile()
    return nc


@functools.lru_cache(maxsize=1)
def _get_program():
    return _build_program()


def _make_in_maps(queries, keys, values, Wq, Wk, Wv, Wo, bo):
    q = np.asarray(queries, np.float32).reshape(S, D)
    kT = np.ascontiguousarray(np.asarray(keys, np.float32).reshape(S, D).T
                              ).astype(ml_dtypes.bfloat16)
    vT = np.ascontiguousarray(np.asarray(values, np.float32).reshape(S, D).T
                              ).astype(ml_dtypes.bfloat16)
    Wq = np.ascontiguousarray(np.asarray(Wq, np.float32)).astype(ml_dtypes.bfloat16)
    Wk = np.ascontiguousarray(np.asarray(Wk, np.float32)).astype(ml_dtypes.bfloat16)
    Wv = np.ascontiguousarray(np.asarray(Wv, np.float32)).astype(ml_dtypes.bfloat16)
    Wo = np.ascontiguousarray(np.asarray(Wo, np.float32)).astype(ml_dtypes.bfloat16)
    bo = np.ascontiguousarray(np.asarray(bo, np.float32))
    in_maps = []
    for c in range(NCORES):
        in_maps.append({
            "xqT": np.ascontiguousarray(q[c * CH:(c + 1) * CH].T).astype(ml_dtypes.bfloat16),
            "keysT": kT, "valsT": vT,
            "Wq": Wq, "Wk": Wk, "Wv": Wv, "Wo": Wo, "bo": bo,
        })
    return in_maps


def _run(in_maps, **kw):
    nc = _get_program()
    return run_bass_kernel_spmd(nc, in_maps, core_ids=list(range(NCORES)), **kw)


def kernel(queries, keys, values, Wq, Wk, Wv, Wo, bo):
    res = _run(_make_in_maps(queries, keys, values, Wq, Wk, Wv, Wo, bo))
    out = np.concatenate([res.results[c]["y"] for c in range(NCORES)], axis=0)
    return out.reshape(1, S, D)


def run_traced(queries, keys, values, Wq, Wk, Wv, Wo, bo):
    """Like kernel() but with NTFF profiling; returns (output, BassKernelResults)."""
    import types
    import trn_agent_boot.trn_boot as _tb
    from concourse import bass_utils
    hook = _tb._ntff_profile_via_ctypes("/opt/axon/libaxon_pjrt.so")
    mod = types.ModuleType("antenv.axon_hooks")
    mod.get_axon_ntff_profile_hook = lambda: hook
    sys.modules["antenv.axon_hooks"] = mod
    bass_utils.upload_artifacts = lambda tmpdir: tmpdir
    res = _run(_make_in_maps(queries, keys, values, Wq, Wk, Wv, Wo, bo), trace=True)
    out = np.concatenate([res.results[c]["y"] for c in range(NCORES)], axis=0)
    return out.reshape(1, S, D), res


# revision 18
# speedup vs baseline: 1.0595x; 1.0031x over previous
"""Trainium2 Bass kernel: MultiHeadSelfAttention (B=1, S=4096, D=512, H=8, DK=DV=64)
with fc_out applied twice.

Sharding: sequence-sharded across 8 cores (512 queries per core). Every core
receives the FULL keys/values (pre-transposed, bf16) and redundantly computes
the full K^T / V projections on-device (cheaper than an AllGather, whose entry
barrier + transfer measured ~100us); attention + the two output projections run
on the core's own 512-query chunk. Host concatenates the 8 output chunks.

Layout notes:
  - scores^T tiles [seq_k(128) x seq_q(512)] come out of PE via lhsT=K^T block,
    rhs=q^T. Both are zero-padded from d=64 to K=128 partitions: K=64 matmuls
    never trip the PE HAM activity monitor, pinning the clock to 1.2 GHz
    (measured); K=128 with zero rows sustains 2.4 GHz.
  - softmax denominator via a ones-column appended to each head's V (stride
    65): attn@V gives [65, 512] per head = output^T rows + exp-sum row.
"""
import sys, functools
sys.path.insert(0, "/opt/trn_rl_repo")
if "/root/.axon_site" not in sys.path:
    sys.path.insert(0, "/root/.axon_site")
import numpy as np
import ml_dtypes

import concourse.bass as bass
import concourse.tile as tile
from concourse import bacc, mybir, masks
from concourse.bass_utils import run_bass_kernel_spmd

NCORES = 8
S, D, H, DK = 4096, 512, 8, 64
CH = S // NCORES            # 512 sequence rows per core
VW = H * (DK + 1)           # 520: v row width incl. ones columns
JT = S // 128               # 32 seq_k tiles
CHUNK = 3                   # j-tiles per exp batch ([128,1536] psum, 3 banks x 2)

F32 = mybir.dt.float32
BF16 = mybir.dt.bfloat16
EXP = mybir.ActivationFunctionType.Exp


def _build_program():
    nc = bacc.Bacc("TRN2", target_bir_lowering=False, debug=False,
                   num_devices=NCORES)

    xqT = nc.dram_tensor("xqT", [D, CH], BF16, kind="ExternalInput")
    keysT = nc.dram_tensor("keysT", [D, S], BF16, kind="ExternalInput")
    valsT = nc.dram_tensor("valsT", [D, S], BF16, kind="ExternalInput")
    Wq = nc.dram_tensor("Wq", [D, D], BF16, kind="ExternalInput")
    Wk = nc.dram_tensor("Wk", [D, D], BF16, kind="ExternalInput")
    Wv = nc.dram_tensor("Wv", [D, D], BF16, kind="ExternalInput")
    Wo = nc.dram_tensor("Wo", [D, D], BF16, kind="ExternalInput")
    bo = nc.dram_tensor("bo", [D], F32, kind="ExternalInput")
    y = nc.dram_tensor("y", [CH, D], F32, kind="ExternalOutput")

    with tile.TileContext(nc) as tc:
        with tc.tile_pool(name="persist", bufs=1) as pp, \
             tc.tile_pool(name="kv", bufs=1) as kvp:

            Wo_sb = pp.tile([128, 2048], BF16, tag="wo")
            Wk_sb = pp.tile([128, 2048], BF16, tag="wk")
            Wv_sb = pp.tile([128, 2048], BF16, tag="wv")
            bo_sb = pp.tile([128, 4], F32, tag="bo")
            ident = pp.tile([128, 128], BF16, tag="id")
            # q^T per head: even heads rows 0-63 (zeros below), odd heads rows
            # 64-127 (zeros above) - matches the packed K^T pair layout
            qTz_sb = pp.tile([128, H * 512], BF16, tag="qt")
            attT_sb = pp.tile([128, 2048], BF16, tag="att")
            # K^T packed head pairs: head 2p on rows 0-63, head 2p+1 on 64-127;
            # the zero padding that keeps scores at K=128 lives in qTz instead
            KTp = [kvp.tile([128, S], BF16, tag=f"kt{p}", name=f"KT{p}")
                   for p in range(H // 2)]
            # V natural [seq, head-stripes of 65 (64 + ones col)]
            V_sb = kvp.tile([128, JT * VW], BF16, tag="v")

            masks.make_identity(nc, ident[:])
            # zero pads + ones columns on gpsimd (keeps DVE free)
            nc.vector.memset(qTz_sb[:], 0.0)
            nc.gpsimd.memset(
                V_sb[:].rearrange("p (j h x) -> p j h x", j=JT, h=H, x=DK + 1)
                [:, :, :, DK:DK + 1], 1.0)

            with tc.tile_pool(name="kstage", bufs=1) as ksp, \
                 tc.tile_pool(name="pt", bufs=6) as ptp, \
                 tc.tile_pool(name="rc", bufs=2) as rcp, \
                 tc.tile_pool(name="ps_av", bufs=1, space="PSUM") as psav:

                def staged_load(dst_sb, src_dram, nchunks=8):
                    w = S // nchunks
                    dst = dst_sb[:].rearrange("p (k s) -> p k s", k=4)
                    srcv = src_dram.ap().rearrange("(k p) s -> p k s", p=128)
                    for ci in range(nchunks):
                        nc.sync.dma_start(dst[:, :, w * ci:w * ci + w],
                                          srcv[:, :, w * ci:w * ci + w])

                def q_proj(pool):
                    for m in range(4):
                        ps = pool.tile([128, 512], F32, tag="bg", name=f"qp{m}")
                        for k in range(4):
                            nc.tensor.matmul(
                                ps[:], lhsT=Wq_sb[:, 512 * k + 128 * m:512 * k + 128 * m + 128],
                                rhs=xqT_sb[:, 512 * k:512 * k + 512],
                                start=(k == 0), stop=(k == 3))
                        nc.vector.tensor_copy(
                            qTz_sb[0:64, 512 * (2 * m):512 * (2 * m) + 512], ps[0:64, :])
                        nc.vector.tensor_copy(
                            qTz_sb[64:128, 512 * (2 * m + 1):512 * (2 * m + 1) + 512],
                            ps[64:128, :])

                def v_proj_group(j, pool):
                    ps = pool.tile([128, 512], F32, tag="bg", name=f"vp{j}")
                    for k in range(4):
                        nc.tensor.matmul(
                            ps[:], lhsT=vst[:, S * k + 128 * j:S * k + 128 * j + 128],
                            rhs=Wv_sb[:, 512 * k:512 * k + 512],
                            start=(k == 0), stop=(k == 3))
                    dst = V_sb[:, VW * j:VW * j + VW].rearrange(
                        "p (h x) -> p h x", h=H, x=DK + 1)[:, :, 0:DK]
                    nc.vector.tensor_copy(
                        dst, ps[:].rearrange("p (h x) -> p h x", h=H, x=DK))

                def k_proj_group(m, sc, kst, pool, tag):
                    ps = pool.tile([128, 512], F32, tag=tag, name=f"kp{m}_{sc}")
                    for k in range(4):
                        nc.tensor.matmul(
                            ps[:], lhsT=Wk_sb[:, 512 * k + 128 * m:512 * k + 128 * m + 128],
                            rhs=kst[:, S * k + 512 * sc:S * k + 512 * sc + 512],
                            start=(k == 0), stop=(k == 3))
                    nc.vector.tensor_copy(KTp[m][:, 512 * sc:512 * sc + 512], ps[:])

                def attention_head(h, pool_sc, chunk, drip=None):
                    q_ap = qTz_sb[:, 512 * h:512 * h + 512]
                    av = psav.tile([65, 512], F32, tag="av", name=f"av{h}")

                    def attn_v(js, pt):
                        for i, j in enumerate(js):
                            nc.tensor.matmul(
                                av[:],
                                lhsT=V_sb[:, VW * j + 65 * h:VW * j + 65 * h + 65],
                                rhs=pt[:, 512 * i:512 * i + 512],
                                start=(j == 0), stop=(j == JT - 1))

                    pend = None  # attn@V lags one chunk so scores stay ahead of ACT
                    for c in range((JT + chunk - 1) // chunk):
                        js = list(range(chunk * c, min(chunk * c + chunk, JT)))
                        ps = pool_sc.tile([128, 512 * chunk], F32, tag="sc",
                                          name=f"sc{h}_{c}")
                        pt = ptp.tile([128, 512 * chunk], BF16, tag="pt",
                                      name=f"pt{h}_{c}")
                        for i, j in enumerate(js):
                            nc.tensor.matmul(
                                ps[:, 512 * i:512 * i + 512],
                                lhsT=KTp[h // 2][:, 128 * j:128 * j + 128],
                                rhs=q_ap, start=True, stop=True)
                        w = 512 * len(js)
                        nc.scalar.activation(pt[:, 0:w], ps[:, 0:w], EXP, scale=0.125)
                        if drip is not None:
                            drip(c)
                        if pend is not None:
                            attn_v(*pend)
                        pend = (js, pt)
                    attn_v(*pend)
                    hp, hl = h // 2, h % 2
                    # copy psum accumulator out immediately so the bank frees
                    avc = rcp.tile([65, 512], F32, tag="avc", name=f"avc{h}")
                    rbc = rcp.tile([64, 512], F32, tag="rb", name=f"rb{h}")
                    rtmp = rcp.tile([1, 512], F32, tag="rt", name=f"rt{h}")
                    nc.vector.tensor_copy(avc[:], av[:])
                    nc.vector.tensor_copy(rtmp[:], av[64:65, :])
                    rb2 = rcp.tile([64, 512], F32, tag="rb2", name=f"rb2{h}")
                    nc.gpsimd.partition_broadcast(rbc[:], rtmp[:])
                    nc.vector.reciprocal_approx_fast(out=rb2[:], in_=rbc[:])
                    nc.vector.tensor_mul(
                        attT_sb[64 * hl:64 * hl + 64, 512 * hp:512 * hp + 512],
                        avc[0:64, :], rb2[:])

                # ---- scope A: q proj, K0, head 0 with V-proj dripped in ----
                pfx = tc.tile_pool(name="xin", bufs=1)
                xp = pfx.__enter__()
                scA = tc.tile_pool(name="ps_scA", bufs=2, space="PSUM")
                pssc2 = scA.__enter__()
                bgp_cm = tc.tile_pool(name="ps_bg", bufs=2, space="PSUM")
                bgp = bgp_cm.__enter__()

                Wq_sb = xp.tile([128, 2048], BF16, tag="wq")
                xqT_sb = xp.tile([128, 2048], BF16, tag="xq")
                vst = xp.tile([128, 4 * S], BF16, tag="vs", name="vstage")
                kst0 = ksp.tile([128, 4 * S], BF16, tag="ks", name="kstage0")
                nc.sync.dma_start(
                    xqT_sb[:].rearrange("p (k s) -> p k s", k=4),
                    xqT.ap().rearrange("(k p) s -> p k s", p=128))
                nc.sync.dma_start(
                    Wq_sb[:].rearrange("p (k n) -> p k n", k=4),
                    Wq.ap().rearrange("(k p) n -> p k n", p=128))
                nc.sync.dma_start(
                    Wk_sb[:].rearrange("p (k n) -> p k n", k=4),
                    Wk.ap().rearrange("(k p) n -> p k n", p=128))
                staged_load(kst0, keysT)
                nc.sync.dma_start(
                    Wv_sb[:].rearrange("p (k n) -> p k n", k=4),
                    Wv.ap().rearrange("(k p) n -> p k n", p=128))
                staged_load(vst, valsT)
                nc.sync.dma_start(
                    Wo_sb[:].rearrange("p (k n) -> p k n", k=4),
                    Wo.ap().rearrange("(k p) n -> p k n", p=128))
                nc.sync.dma_start(bo_sb[:], bo.ap().rearrange("(m p) -> p m", p=128))

                q_proj(bgp)
                for sc in range(8):
                    k_proj_group(0, sc, kst0, bgp, "bg")
                kst1 = ksp.tile([128, 4 * S], BF16, tag="ks", name="kstage1")
                staged_load(kst1, keysT)
                v_proj_group(0, bgp)
                v_proj_group(1, bgp)

                def drip_v(c):
                    for j in (2 * c + 2, 2 * c + 3):
                        if j < JT:
                            v_proj_group(j, bgp)

                attention_head(0, pssc2, 2, drip_v)

                bgp_cm.__exit__(None, None, None)
                scA.__exit__(None, None, None)
                pfx.__exit__(None, None, None)

                # ---- scope B: heads 1-7, K pairs 1-3 dripped in ----
                with tc.tile_pool(name="ps_sc", bufs=2, space="PSUM") as pssc, \
                     tc.tile_pool(name="ps_kp", bufs=1, space="PSUM") as pskp:
                    ksts = {1: kst1}

                    def make_drip_k(m):
                        def drip(c):
                            if c < 8:
                                k_proj_group(m, c, ksts[m], pskp, "kp")
                        return drip

                    attention_head(1, pssc, CHUNK, make_drip_k(1))
                    ksts[2] = ksp.tile([128, 4 * S], BF16, tag="ks", name="kstage2")
                    staged_load(ksts[2], keysT)
                    attention_head(2, pssc, CHUNK)
                    attention_head(3, pssc, CHUNK, make_drip_k(2))
                    ksts[3] = ksp.tile([128, 4 * S], BF16, tag="ks", name="kstage3")
                    staged_load(ksts[3], keysT)
                    attention_head(4, pssc, CHUNK)
                    attention_head(5, pssc, CHUNK, make_drip_k(3))
                    attention_head(6, pssc, CHUNK)
                    attention_head(7, pssc, CHUNK)

            # ---- fc_out twice (fp32), then transpose to natural layout ----
            with tc.tile_pool(name="fc", bufs=3) as fcp, \
                 tc.tile_pool(name="ps_fc", bufs=2, space="PSUM") as psfc, \
                 tc.tile_pool(name="ps_tr", bufs=2, space="PSUM") as pstr:
                o1T = fcp.tile([128, 2048], BF16, tag="fcb", name="o1T")
                o2T = fcp.tile([128, 2048], BF16, tag="fcb", name="o2T")
                for fsrc, fdst in ((attT_sb, o1T), (o1T, o2T)):
                    for m in range(4):
                        ps = psfc.tile([128, 512], F32, tag="fc")
                        for k in range(4):
                            nc.tensor.matmul(
                                ps[:], lhsT=Wo_sb[:, 512 * k + 128 * m:512 * k + 128 * m + 128],
                                rhs=fsrc[:, 512 * k:512 * k + 512],
                                start=(k == 0), stop=(k == 3))
                        nc.vector.tensor_scalar_add(
                            fdst[:, 512 * m:512 * m + 512], ps[:], bo_sb[:, m:m + 1])
                onat = fcp.tile([128, 2048], F32, tag="fcb", name="onat")
                for kf in range(4):
                    for m in range(4):
                        pst = pstr.tile([128, 128], BF16, tag="tr")
                        nc.tensor.transpose(
                            pst[:], o2T[:, 512 * kf + 128 * m:512 * kf + 128 * m + 128],
                            ident[:])
                        nc.vector.tensor_copy(
                            onat[:, 512 * m + 128 * kf:512 * m + 128 * kf + 128],
                            pst[:])
                nc.sync.dma_start(
                    y.ap().rearrange("(m p) f -> p m f", m=4, p=128),
                    onat[:].rearrange("p (m f) -> p m f", m=4))

    nc.compile()
    return nc


@functools.lru_cache(maxsize=1)
def _get_program():
    return _build_program()


def _make_in_maps(queries, keys, values, Wq, Wk, Wv, Wo, bo):
    q = np.asarray(queries, np.float32).reshape(S, D)
    kT = np.ascontiguousarray(np.asarray(keys, np.float32).reshape(S, D).T
                              ).astype(ml_dtypes.bfloat16)
    vT = np.ascontiguousarray(np.asarray(values, np.float32).reshape(S, D).T
                              ).astype(ml_dtypes.bfloat16)
    Wq = np.ascontiguousarray(np.asarray(Wq, np.float32)).astype(ml_dtypes.bfloat16)
    Wk = np.ascontiguousarray(np.asarray(Wk, np.float32)).astype(ml_dtypes.bfloat16)
    Wv = np.ascontiguousarray(np.asarray(Wv, np.float32)).astype(ml_dtypes.bfloat16)
    Wo = np.ascontiguousarray(np.asarray(Wo, np.float32)).astype(ml_dtypes.bfloat16)
    bo = np.ascontiguousarray(np.asarray(bo, np.float32))
    in_maps = []
    for c in range(NCORES):
        in_maps.append({
            "xqT": np.ascontiguousarray(q[c * CH:(c + 1) * CH].T).astype(ml_dtypes.bfloat16),
            "keysT": kT, "valsT": vT,
            "Wq": Wq, "Wk": Wk, "Wv": Wv, "Wo": Wo, "bo": bo,
        })
    return in_maps


def _run(in_maps, **kw):
    nc = _get_program()
    return run_bass_kernel_spmd(nc, in_maps, core_ids=list(range(NCORES)), **kw)


def kernel(queries, keys, values, Wq, Wk, Wv, Wo, bo):
    res = _run(_make_in_maps(queries, keys, values, Wq, Wk, Wv, Wo, bo))
    out = np.concatenate([res.results[c]["y"] for c in range(NCORES)], axis=0)
    return out.reshape(1, S, D)


def run_traced(queries, keys, values, Wq, Wk, Wv, Wo, bo):
    """Like kernel() but with NTFF profiling; returns (output, BassKernelResults)."""
    import types
    import trn_agent_boot.trn_boot as _tb
    from concourse import bass_utils
    hook = _tb._ntff_profile_via_ctypes("/opt/axon/libaxon_pjrt.so")
    mod = types.ModuleType("antenv.axon_hooks")
    mod.get_axon_ntff_profile_hook = lambda: hook
    sys.modules["antenv.axon_hooks"] = mod
    bass_utils.upload_artifacts = lambda tmpdir: tmpdir
    res = _run(_make_in_maps(queries, keys, values, Wq, Wk, Wv, Wo, bo), trace=True)
    out = np.concatenate([res.results[c]["y"] for c in range(NCORES)], axis=0)
    return out.reshape(1, S, D), res
